# revision 6
# baseline (speedup 1.0000x reference)
"""Trainium2 Bass kernel for nn_Conv2d_24833500905755.

Computes the reference's "mismatched flatten order" conv:
  out[b,co,h,w] = sum_{c,di,dj} xpad[b,c,h+di,w+dj] * Wt[c, di*3+dj, co]
with Wt = K.reshape(576, C_OUT).reshape(C_IN, 9, C_OUT).

Strategy (data-parallel over 8 cores, 4 images per core):
  - Host: scramble K into Wt, shard x on batch, replicate Wt.
  - Core: pack 2 images on the 128-partition dim (C_IN=64 each half).
    DMA each image pair into a zero-padded [128, 58, 58] SBUF tile.
    For each 8-row output chunk, accumulate 9 shifted matmuls per
    image half into PSUM (K=64 contraction in partition rows 0-63 /
    64-127 -> concurrent PE row-group tiles), copy PSUM -> SBUF,
    one big DMA per image back to HBM.
"""

import numpy as np

import concourse.bass as bass
import concourse.mybir as mybir
from concourse.bass_utils import run_bass_kernel_spmd
from concourse.tile import TileContext
from concourse.vector_clock import ScopedClock


_WAIT_LIMIT = 1


class PatchedTileContext(TileContext):
    """The container's walrus rejects instructions carrying more than one
    semaphore wait ("Too many sync wait commands"). Hoist excess waits onto
    same-engine NoOps committed just before, and split the kernel-tail Drain
    into a chain of single-wait drains."""

    def _commit_instruction(self, inst, lazy_reg_writes=True):
        si = getattr(inst, "sync_info", None)
        if (
            si is not None
            and si.on_wait is not None
            and len(si.on_wait) > _WAIT_LIMIT
            and inst.engine != mybir.EngineType.Unassigned
        ):
            waits = list(si.on_wait)
            extra, keep = waits[:-_WAIT_LIMIT], waits[-_WAIT_LIMIT:]
            for i in range(0, len(extra), _WAIT_LIMIT):
                noop = mybir.InstNoOp(
                    name=f"{inst.name}_hw{i}",
                    engine=inst.engine,
                    sync_info=mybir.SyncInfo(
                        on_wait=extra[i : i + _WAIT_LIMIT], on_update=[]
                    ),
                    bass_nofuse=True,
                )
                super()._commit_instruction(noop, lazy_reg_writes=False)
            inst.sync_info.on_wait = keep
        return super()._commit_instruction(inst, lazy_reg_writes=lazy_reg_writes)

    def _drain_and_barrier(self, tick_clock, wait_clock):
        nc = self.nc
        drain_inst = nc.sync.drain()
        wait_clock.add_sem_waits(
            drain_inst.ins, ScopedClock({None: tick_clock.global_clock})
        )
        waits = list(drain_inst.ins.sync_info.on_wait)
        if len(waits) > 1:
            drain_inst.ins.sync_info.on_wait = [waits[0]]
            num2handle = {h.num: h for h in self.sems.allocated().values()}
            for w in waits[1:]:
                d2 = nc.sync.drain()
                d2.wait_op(num2handle[w.id], w.wait_value, "sem-ge")
        nc.all_engine_barrier()
        assert self.sems is not None
        popped = nc._tile_sem_poison_stack.pop()
        assert popped is self._sem_poison
        nc.clear_and_free_semaphores(list(self.sems.allocated().values()))
        nc.all_engine_barrier()

B, C_IN, C_OUT, H = 32, 64, 128, 56
KS = 3
N_CORES = 8
BPC = B // N_CORES        # images per core
HP = H + 2               # padded height/width (pad=1)
RCHUNK = 8               # output rows per PSUM tile (8*56=448 <= 512 fp32/bank)
NCHUNK = H // RCHUNK     # 7

# matmul input dtype: float32 (safe) or float32r (4x faster, reduced precision)
MM_DT = mybir.dt.float32


def build_nc(mm_dt=MM_DT):
    f32 = mybir.dt.float32
    nc = bass.Bass()
    x_ext = nc.declare_dram_parameter("x", [BPC, C_IN, H, H], mm_dt, isOutput=False)
    w_ext = nc.declare_dram_parameter("w", [2 * C_IN, KS * KS, C_OUT], mm_dt, isOutput=False)
    out_ext = nc.declare_dram_parameter("out", [BPC, C_OUT, H, H], f32, isOutput=True)

    with PatchedTileContext(nc) as tc:
        with (
            tc.tile_pool(name="wp", bufs=1) as wpool,
            tc.tile_pool(name="xp", bufs=2) as xpool,
            tc.tile_pool(name="op", bufs=2) as opool,
            tc.tile_pool(name="ps", bufs=4, space="PSUM") as pspool,
        ):
            wt = wpool.tile([2 * C_IN, KS * KS, C_OUT], mm_dt)
            nc.sync.dma_start(out=wt[:], in_=w_ext[:])

            for p in range(BPC // 2):  # image pairs
                xp = xpool.tile([2 * C_IN, HP, HP], mm_dt)
                # zero the 1-px border (matmul reads it as conv padding)
                nc.vector.memset(xp[:, 0:1, :], 0.0)
                nc.vector.memset(xp[:, HP - 1 : HP, :], 0.0)
                nc.vector.memset(xp[:, 1 : HP - 1, 0:1], 0.0)
                nc.vector.memset(xp[:, 1 : HP - 1, HP - 1 : HP], 0.0)
                src = x_ext[2 * p : 2 * p + 2].rearrange("b c h w -> (b c) h w")
                # split the interior load so early chunks unblock sooner
                nc.sync.dma_start(out=xp[:, 1:29, 1 : HP - 1], in_=src[:, 0:28, :])
                nc.sync.dma_start(out=xp[:, 29:57, 1 : HP - 1], in_=src[:, 28:56, :])

                obs = [opool.tile([C_OUT, H, H], f32, tag=f"ob{i}", name=f"ob{i}_{p}") for i in range(2)]
                for ci in range(NCHUNK):
                    h0 = ci * RCHUNK
                    pss = [pspool.tile([C_OUT, RCHUNK, H], f32, tag=f"ps{j}", name=f"ps{j}_{p}_{ci}") for j in range(2)]
                    for k in range(KS * KS):
                        di, dj = divmod(k, KS)
                        for half in range(2):
                            c0 = half * C_IN
                            nc.tensor.matmul(
                                out=pss[half][:],
                                lhsT=wt[c0 : c0 + C_IN, k, :],
                                rhs=xp[c0 : c0 + C_IN, h0 + di : h0 + di + RCHUNK, dj : dj + H],
                                start=(k == 0),
                                stop=(k == KS * KS - 1),
                            )
                    for half in range(2):
                        nc.any.tensor_copy(out=obs[half][:, h0 : h0 + RCHUNK, :], in_=pss[half][:])
                for half in range(2):
                    dst = out_ext[2 * p + half : 2 * p + half + 1].rearrange(
                        "b c h w -> (b c) h w"
                    )
                    nc.sync.dma_start(out=dst, in_=obs[half][:])
    return nc


def _prep_inputs(x, K):
    x = np.ascontiguousarray(np.asarray(x, dtype=np.float32))
    K = np.ascontiguousarray(np.asarray(K, dtype=np.float32))
    Wt = K.reshape(KS * KS * C_IN, C_OUT).reshape(C_IN, KS * KS, C_OUT)
    Wrep = np.ascontiguousarray(np.concatenate([Wt, Wt], axis=0))  # [128, 9, C_OUT]
    shards = x.reshape(N_CORES, BPC, C_IN, H, H)
    return [{"x": np.ascontiguousarray(shards[i]), "w": Wrep} for i in range(N_CORES)]


def run(x, K, trace=False, mm_dt=MM_DT):
    nc = build_nc(mm_dt)
    in_maps = _prep_inputs(x, K)
    res = run_bass_kernel_spmd(nc, in_maps, list(range(N_CORES)), trace=trace)
    out = np.concatenate([res.results[i]["out"] for i in range(N_CORES)], axis=0)
    return out, res


def kernel(x, K):
    out, _ = run(x, K, trace=False)
    return out


# revision 12
# speedup vs baseline: 2.4480x; 2.4480x over previous
"""Trainium2 Bass kernel for nn_Conv2d_24833500905755.

Computes the reference's "mismatched flatten order" conv:
  out[b,co,h,w] = sum_{c,di,dj} xpad[b,c,h+di,w+dj] * Wt[c, di*3+dj, co]
with Wt = K.reshape(576, C_OUT).reshape(C_IN, 9, C_OUT).

Strategy (data-parallel over 8 cores, 4 images per core):
  - Host: scramble K into Wt, shard x on batch, replicate Wt.
  - Core: pack 2 images on the 128-partition dim (C_IN=64 each half).
    DMA each image pair into a zero-padded [128, 58, 58] SBUF tile.
    For each 8-row output chunk, accumulate 9 shifted matmuls per
    image half into PSUM (K=64 contraction in partition rows 0-63 /
    64-127 -> concurrent PE row-group tiles), copy PSUM -> SBUF,
    one big DMA per image back to HBM.
"""

import numpy as np

import concourse.bass as bass
import concourse.mybir as mybir
from concourse.bass_utils import run_bass_kernel_spmd
from concourse.tile import TileContext
from concourse.vector_clock import ScopedClock


_WAIT_LIMIT = 1


class PatchedTileContext(TileContext):
    """The container's walrus rejects instructions carrying more than one
    semaphore wait ("Too many sync wait commands"). Hoist excess waits onto
    same-engine NoOps committed just before, and split the kernel-tail Drain
    into a chain of single-wait drains."""

    def _commit_instruction(self, inst, lazy_reg_writes=True):
        si = getattr(inst, "sync_info", None)
        if (
            si is not None
            and si.on_wait is not None
            and len(si.on_wait) > _WAIT_LIMIT
            and inst.engine != mybir.EngineType.Unassigned
        ):
            waits = list(si.on_wait)
            extra, keep = waits[:-_WAIT_LIMIT], waits[-_WAIT_LIMIT:]
            for i in range(0, len(extra), _WAIT_LIMIT):
                noop = mybir.InstNoOp(
                    name=f"{inst.name}_hw{i}",
                    engine=inst.engine,
                    sync_info=mybir.SyncInfo(
                        on_wait=extra[i : i + _WAIT_LIMIT], on_update=[]
                    ),
                    bass_nofuse=True,
                )
                super()._commit_instruction(noop, lazy_reg_writes=False)
            inst.sync_info.on_wait = keep
        return super()._commit_instruction(inst, lazy_reg_writes=lazy_reg_writes)

    def _drain_and_barrier(self, tick_clock, wait_clock):
        nc = self.nc
        drain_inst = nc.sync.drain()
        wait_clock.add_sem_waits(
            drain_inst.ins, ScopedClock({None: tick_clock.global_clock})
        )
        waits = list(drain_inst.ins.sync_info.on_wait)
        if len(waits) > 1:
            drain_inst.ins.sync_info.on_wait = [waits[0]]
            num2handle = {h.num: h for h in self.sems.allocated().values()}
            for w in waits[1:]:
                d2 = nc.sync.drain()
                d2.wait_op(num2handle[w.id], w.wait_value, "sem-ge")
        nc.all_engine_barrier()
        assert self.sems is not None
        popped = nc._tile_sem_poison_stack.pop()
        assert popped is self._sem_poison
        nc.clear_and_free_semaphores(list(self.sems.allocated().values()))
        nc.all_engine_barrier()

B, C_IN, C_OUT, H = 32, 64, 128, 56
KS = 3
N_CORES = 8
BPC = B // N_CORES        # images per core
HP = H + 2               # padded height/width (pad=1)
RCHUNK = 8               # output rows per PSUM tile (8*56=448 <= 512 fp32/bank)
NCHUNK = H // RCHUNK     # 7

# matmul input dtype: float32 (safe) or float32r (4x faster, ~1.4e-4 rel err)
MM_DT = mybir.dt.float32r


def build_nc(mm_dt=MM_DT):
    f32 = mybir.dt.float32
    nc = bass.Bass()
    # x arrives pre-padded (1-px zero border) from the host
    x_ext = nc.declare_dram_parameter("x", [BPC, C_IN, HP, HP], mm_dt, isOutput=False)
    w_ext = nc.declare_dram_parameter("w", [2 * C_IN, KS * KS, C_OUT], mm_dt, isOutput=False)
    out_ext = nc.declare_dram_parameter("out", [BPC, C_OUT, H, H], f32, isOutput=True)

    with PatchedTileContext(nc) as tc:
        with (
            tc.tile_pool(name="wp", bufs=1) as wpool,
            tc.tile_pool(name="xp", bufs=2) as xpool,
            tc.tile_pool(name="op", bufs=2) as opool,
            tc.tile_pool(name="ps", bufs=4, space="PSUM") as pspool,
        ):
            wt = wpool.tile([2 * C_IN, KS * KS, C_OUT], mm_dt)
            nc.sync.dma_start(out=wt[:], in_=w_ext[:])

            # output staging blocks (DMA'd out as soon as filled): rows
            # [0,24) after chunk 2, [24,56) after chunk 6
            OBLOCKS = [(0, 24), (24, 56)]
            for p in range(BPC // 2):  # image pairs
                xp = xpool.tile([2 * C_IN, HP, HP], mm_dt)
                src = x_ext[2 * p : 2 * p + 2].rearrange("b c h w -> (b c) h w")
                # split the load so early chunks unblock sooner
                nc.sync.dma_start(out=xp[:, 0:12, :], in_=src[:, 0:12, :])
                nc.sync.dma_start(out=xp[:, 12:34, :], in_=src[:, 12:34, :])
                nc.sync.dma_start(out=xp[:, 34:HP, :], in_=src[:, 34:HP, :])

                for ci in range(NCHUNK):
                    h0 = ci * RCHUNK
                    blo, bhi = next(b for b in OBLOCKS if b[0] <= h0 < b[1])
                    if h0 == blo:
                        obs = [
                            opool.tile([C_OUT, bhi - blo, H], f32, tag=f"ob{i}",
                                       name=f"ob{i}_{p}_{h0}")
                            for i in range(2)
                        ]
                    pss = [pspool.tile([C_OUT, RCHUNK, H], f32, tag=f"ps{j}", name=f"ps{j}_{p}_{ci}") for j in range(2)]
                    for k in range(KS * KS):
                        di, dj = divmod(k, KS)
                        for half in range(2):
                            c0 = half * C_IN
                            nc.tensor.matmul(
                                out=pss[half][:],
                                lhsT=wt[c0 : c0 + C_IN, k, :],
                                rhs=xp[c0 : c0 + C_IN, h0 + di : h0 + di + RCHUNK, dj : dj + H],
                                start=(k == 0),
                                stop=(k == KS * KS - 1),
                            )
                    for half in range(2):
                        nc.any.tensor_copy(
                            out=obs[half][:, h0 - blo : h0 - blo + RCHUNK, :],
                            in_=pss[half][:],
                        )
                    if h0 + RCHUNK == bhi:
                        for half in range(2):
                            dst = out_ext[2 * p + half : 2 * p + half + 1].rearrange(
                                "b c h w -> (b c) h w"
                            )
                            # scalar-engine HWDGE ring: keeps Sync free for loads
                            nc.scalar.dma_start(
                                out=dst[:, blo:bhi, :], in_=obs[half][:]
                            )
    return nc


def _prep_inputs(x, K):
    x = np.ascontiguousarray(np.asarray(x, dtype=np.float32))
    K = np.ascontiguousarray(np.asarray(K, dtype=np.float32))
    xpad = np.pad(x, ((0, 0), (0, 0), (1, 1), (1, 1)))
    Wt = K.reshape(KS * KS * C_IN, C_OUT).reshape(C_IN, KS * KS, C_OUT)
    Wrep = np.ascontiguousarray(np.concatenate([Wt, Wt], axis=0))  # [128, 9, C_OUT]
    shards = xpad.reshape(N_CORES, BPC, C_IN, HP, HP)
    return [{"x": np.ascontiguousarray(shards[i]), "w": Wrep} for i in range(N_CORES)]


def run(x, K, trace=False, mm_dt=MM_DT):
    nc = build_nc(mm_dt)
    in_maps = _prep_inputs(x, K)
    res = run_bass_kernel_spmd(nc, in_maps, list(range(N_CORES)), trace=trace)
    out = np.concatenate([res.results[i]["out"] for i in range(N_CORES)], axis=0)
    return out, res


def kernel(x, K):
    out, _ = run(x, K, trace=False)
    return out
